# revision 3
# baseline (speedup 1.0000x reference)
"""Kohonen map / VQ codebook kernel for 8 Trainium2 NeuronCores.

dist[n,k] = ||x_n - w_k||^2 ; out[n] = w[argmin_k dist]
argmin_k dist = argmax_k ( 2*x.w_k - ||w_k||^2 )  (||x||^2 const per row)

Centering trick: dist is invariant under x->x-c, w->w-c (c=0.5 vector), and
centered operands shrink float32r (TF32-ish) matmul rounding ~4x.

Per core (data-parallel over N): x shard [8192, 512] passed transposed as
xt [512, 8192]; codebook replicated: wt2 = 2*(w-0.5)^T as matmul rhs,
nw2b = -||w-0.5||^2 broadcast for ACT->PSUM prefill, w original for the
exact row gather via indirect DMA.
"""
import os
import sys

sys.path.insert(0, "/opt/trn_rl_repo")

import numpy as np

import concourse.bass as bass
import concourse.bacc as bacc
import concourse.mybir as mybir
from concourse.tile import TileContext
from concourse.bass_utils import run_bass_kernel_spmd

f32 = mybir.dt.float32
f32r = mybir.dt.float32r
u32 = mybir.dt.uint32

N, K, D = 65536, 4096, 512
NCORES = 8
NL = N // NCORES          # 8192 rows per core
P = 128                   # partitions
NT = NL // P              # 64 n-tiles per core
DC = D // P               # 4 contraction chunks
KC = K // 512             # 8 psum-bank chunks

_LAST_RESULTS = None      # test.py reads exec_time_ns from here


def _build():
    nc = bacc.Bacc(None, target_bir_lowering=False)
    xt = nc.dram_tensor("xt", [D, NL], f32, kind="ExternalInput")
    wt2 = nc.dram_tensor("wt2", [D, K], f32, kind="ExternalInput")
    nw2b = nc.dram_tensor("nw2b", [P, K], f32, kind="ExternalInput")
    w = nc.dram_tensor("w", [K, D], f32, kind="ExternalInput")
    y = nc.dram_tensor("y", [NL, D], f32, kind="ExternalOutput")

    with TileContext(nc) as tc:
        with tc.tile_pool(name="const", bufs=1) as cp, \
             tc.tile_pool(name="xp", bufs=12) as xp, \
             tc.tile_pool(name="sp", bufs=3) as sp, \
             tc.tile_pool(name="mp", bufs=4) as mp, \
             tc.tile_pool(name="gp", bufs=4) as gp, \
             tc.tile_pool(name="pp", bufs=8, space="PSUM") as pp:
            wt_sb = []
            for c in range(DC):
                t = cp.tile([P, K], f32, tag=f"wt{c}")
                nc.sync.dma_start(out=t[:], in_=wt2[c * P:(c + 1) * P, :])
                wt_sb.append(t)
            nw2_sb = cp.tile([P, K], f32, tag="nw2")
            nc.sync.dma_start(out=nw2_sb[:], in_=nw2b[:, :])
            zrow = cp.tile([1, 512], f32, tag="zrow")
            nc.vector.memset(zrow[:], 0.0)
            # warm every PSUM slot: start=True rank-1 zero matmul sets all
            # has_written bits so later start=False groups accumulate onto
            # the ScalarE -|w|^2 prefill instead of overwriting it.
            for _ in range(KC):
                pw = pp.tile([P, 512], f32, tag="ps")
                nc.tensor.matmul(out=pw[:], lhsT=zrow[0:1, 0:P], rhs=zrow[0:1, :],
                                 start=True, stop=True, skip_group_check=True)

            for nt in range(NT):
                xts = []
                for c in range(DC):
                    t = xp.tile([P, P], f32, tag="xt")
                    nc.sync.dma_start(
                        out=t[:], in_=xt[c * P:(c + 1) * P, nt * P:(nt + 1) * P])
                    xts.append(t)
                s_sb = sp.tile([P, K], f32, tag="s")
                for kc in range(KC):
                    ps = pp.tile([P, 512], f32, tag="ps")
                    ksl = slice(kc * 512, (kc + 1) * 512)
                    # prefill -|w|^2 via ScalarE; matmuls accumulate on top
                    nc.scalar.copy(out=ps[:], in_=nw2_sb[:, ksl])
                    for c in range(DC):
                        nc.tensor.matmul(out=ps[:], lhsT=xts[c][:],
                                         rhs=wt_sb[c][:, ksl],
                                         start=False, stop=(c == DC - 1),
                                         skip_group_check=True)
                    nc.scalar.copy(out=s_sb[:, ksl], in_=ps[:])
                mx8 = mp.tile([P, 8], f32, tag="mx")
                idx8 = mp.tile([P, 8], u32, tag="idx")
                nc.vector.max(out=mx8[:], in_=s_sb[:])
                nc.vector.max_index(out=idx8[:], in_max=mx8[:], in_values=s_sb[:])
                wsel = gp.tile([P, D], f32, tag="wsel")
                nc.gpsimd.indirect_dma_start(
                    out=wsel[:], out_offset=None, in_=w[:, :],
                    in_offset=bass.IndirectOffsetOnAxis(ap=idx8[:, :1], axis=0))
                nc.sync.dma_start(out=y[nt * P:(nt + 1) * P, :], in_=wsel[:])
    nc.finalize()
    return nc


def kernel(x, weights):
    global _LAST_RESULTS
    x = np.ascontiguousarray(np.asarray(x, dtype=np.float32))
    weights = np.ascontiguousarray(np.asarray(weights, dtype=np.float32))

    xc = x - np.float32(0.5)
    wc = weights - np.float32(0.5)
    wt2 = np.ascontiguousarray((2.0 * wc).T)                    # [D, K]
    nw2 = -(wc * wc).sum(1, dtype=np.float32)                   # [K]
    nw2b = np.ascontiguousarray(np.broadcast_to(nw2, (P, K)))   # [P, K]

    nc = _build()
    in_maps = []
    for i in range(NCORES):
        xt_i = np.ascontiguousarray(xc[i * NL:(i + 1) * NL].T)  # [D, NL]
        in_maps.append(dict(xt=xt_i, wt2=wt2, nw2b=nw2b, w=weights))

    trace = bool(int(os.environ.get("KOHONEN_TRACE", "0")))
    res = run_bass_kernel_spmd(nc, in_maps, core_ids=list(range(NCORES)),
                               trace=trace)
    _LAST_RESULTS = res
    return np.concatenate([res.results[i]["y"] for i in range(NCORES)], axis=0)


# revision 4
# speedup vs baseline: 1.4082x; 1.4082x over previous
"""Kohonen map / VQ codebook kernel for 8 Trainium2 NeuronCores.

dist[n,k] = ||x_n - w_k||^2 ; out[n] = w[argmin_k dist]
argmin_k dist = argmax_k ( 2*x.w_k - ||w_k||^2 )  (||x||^2 const per row)

Centering trick: dist is invariant under x->x-c, w->w-c (c=0.5 vector), and
centered operands shrink float32r (TF32-ish) matmul rounding ~4x.

Per core (data-parallel over N): x shard [8192, 512] passed transposed as
xt [512, 8192]; codebook replicated: wt2 = 2*(w-0.5)^T as matmul rhs,
nw2b = -||w-0.5||^2 broadcast for ACT->PSUM prefill, w original for the
exact row gather via indirect DMA.
"""
import os
import sys

sys.path.insert(0, "/opt/trn_rl_repo")

import numpy as np

import concourse.bass as bass
import concourse.bacc as bacc
import concourse.mybir as mybir
from concourse.tile import TileContext
from concourse.bass_utils import run_bass_kernel_spmd

f32 = mybir.dt.float32
f32r = mybir.dt.float32r
u32 = mybir.dt.uint32

N, K, D = 65536, 4096, 512
NCORES = 8
NL = N // NCORES          # 8192 rows per core
P = 128                   # partitions
NT = NL // P              # 64 n-tiles per core
DC = D // P               # 4 contraction chunks
KC = K // 512             # 8 psum-bank chunks

_LAST_RESULTS = None      # test.py reads exec_time_ns from here
_NC = None               # cached built+finalized bass module


def _build():
    nc = bacc.Bacc(None, target_bir_lowering=False)
    xt = nc.dram_tensor("xt", [D, NL], f32r, kind="ExternalInput")
    wt2 = nc.dram_tensor("wt2", [D, K], f32r, kind="ExternalInput")
    nw2b = nc.dram_tensor("nw2b", [P, K], f32, kind="ExternalInput")
    w = nc.dram_tensor("w", [K, D], f32, kind="ExternalInput")
    y = nc.dram_tensor("y", [NL, D], f32, kind="ExternalOutput")

    with TileContext(nc) as tc:
        with tc.tile_pool(name="const", bufs=1) as cp, \
             tc.tile_pool(name="xp", bufs=12) as xp, \
             tc.tile_pool(name="sp", bufs=3) as sp, \
             tc.tile_pool(name="mp", bufs=4) as mp, \
             tc.tile_pool(name="gp", bufs=4) as gp, \
             tc.tile_pool(name="pp", bufs=8, space="PSUM") as pp:
            wt_sb = []
            for c in range(DC):
                t = cp.tile([P, K], f32r, tag=f"wt{c}")
                nc.sync.dma_start(out=t[:], in_=wt2[c * P:(c + 1) * P, :])
                wt_sb.append(t)
            nw2_sb = cp.tile([P, K], f32, tag="nw2")
            nc.sync.dma_start(out=nw2_sb[:], in_=nw2b[:, :])
            zrow = cp.tile([1, 512], f32, tag="zrow")
            nc.vector.memset(zrow[:], 0.0)
            # warm every PSUM slot: start=True rank-1 zero matmul sets all
            # has_written bits so later start=False groups accumulate onto
            # the ScalarE -|w|^2 prefill instead of overwriting it.
            for _ in range(KC):
                pw = pp.tile([P, 512], f32, tag="ps")
                nc.tensor.matmul(out=pw[:], lhsT=zrow[0:1, 0:P], rhs=zrow[0:1, :],
                                 start=True, stop=True, skip_group_check=True)

            for nt in range(NT):
                xts = []
                for c in range(DC):
                    t = xp.tile([P, P], f32r, tag="xt")
                    nc.sync.dma_start(
                        out=t[:], in_=xt[c * P:(c + 1) * P, nt * P:(nt + 1) * P])
                    xts.append(t)
                s_sb = sp.tile([P, K], f32, tag="s")
                for kc in range(KC):
                    ps = pp.tile([P, 512], f32, tag="ps")
                    ksl = slice(kc * 512, (kc + 1) * 512)
                    # prefill -|w|^2 via ScalarE; matmuls accumulate on top
                    nc.scalar.copy(out=ps[:], in_=nw2_sb[:, ksl])
                    for c in range(DC):
                        nc.tensor.matmul(out=ps[:], lhsT=xts[c][:],
                                         rhs=wt_sb[c][:, ksl],
                                         start=False, stop=(c == DC - 1),
                                         skip_group_check=True)
                    nc.scalar.copy(out=s_sb[:, ksl], in_=ps[:])
                mx8 = mp.tile([P, 8], f32, tag="mx")
                idx8 = mp.tile([P, 8], u32, tag="idx")
                nc.vector.max(out=mx8[:], in_=s_sb[:])
                nc.vector.max_index(out=idx8[:], in_max=mx8[:], in_values=s_sb[:])
                wsel = gp.tile([P, D], f32, tag="wsel")
                nc.gpsimd.indirect_dma_start(
                    out=wsel[:], out_offset=None, in_=w[:, :],
                    in_offset=bass.IndirectOffsetOnAxis(ap=idx8[:, :1], axis=0))
                nc.sync.dma_start(out=y[nt * P:(nt + 1) * P, :], in_=wsel[:])
    nc.finalize()
    return nc


def kernel(x, weights):
    global _LAST_RESULTS
    x = np.ascontiguousarray(np.asarray(x, dtype=np.float32))
    weights = np.ascontiguousarray(np.asarray(weights, dtype=np.float32))

    xc = x - np.float32(0.5)
    wc = weights - np.float32(0.5)
    wt2 = np.ascontiguousarray((2.0 * wc).T)                    # [D, K]
    nw2 = -(wc * wc).sum(1, dtype=np.float32)                   # [K]
    nw2b = np.ascontiguousarray(np.broadcast_to(nw2, (P, K)))   # [P, K]

    global _NC
    if _NC is None:
        _NC = _build()
    nc = _NC
    in_maps = []
    for i in range(NCORES):
        xt_i = np.ascontiguousarray(xc[i * NL:(i + 1) * NL].T)  # [D, NL]
        in_maps.append(dict(xt=xt_i, wt2=wt2, nw2b=nw2b, w=weights))

    trace = bool(int(os.environ.get("KOHONEN_TRACE", "0")))
    res = run_bass_kernel_spmd(nc, in_maps, core_ids=list(range(NCORES)),
                               trace=trace)
    _LAST_RESULTS = res
    return np.concatenate([res.results[i]["y"] for i in range(NCORES)], axis=0)
